# revision 25
# baseline (speedup 1.0000x reference)
"""Masked dot-product attention on 8 Trainium2 NeuronCores.

Full inputs: queries/keys/values [8, 2048, 128] f32, valid_lens [8] i32.
Output: softmax(Q K^T / sqrt(128), masked to valid_lens) @ V, [8, 2048, 128] f32.

Strategy
--------
Keys at positions >= valid_lens[b] are masked to -1e6 by the reference, so
exp() makes their softmax weight exactly 0: only ceil(vl[b]/128) key-chunks
per batch carry information.  Scores are O(6) in magnitude, so softmax needs
no max-subtraction and partial (numerator, denominator) sums over disjoint
key ranges are additive -- work can be split arbitrarily across cores and
recombined on the host.

The device program (identical on all 8 cores, SPMD) is a flat sequence of
"slots": each slot owns one Q^T tile [128d x 1024q] and C chunk iterations.
Per chunk (128 keys):
  S^T  = K_chunk @ Q^T        two matmuls -> PSUM [128k x 1024q]
  P^T  = exp(scale*S^T + bias[k])  ScalarE, bias is a per-partition mask
                                   (0 valid / -100 masked) -> SBUF bf16
  PV_j += P^T_j^T @ [V_chunk | 1]  8 matmuls (stationary P^T slice, moving
                                   V+ones [128k x 129]) accumulating in PSUM
The ones-column yields the softmax denominator in PSUM column 128.

The exp stream on ScalarE (~1.0us per chunk) is the steady-state bottleneck;
the PE stream is interleaved at CHUNK granularity so the in-order tensor
engine never builds a PV backlog: per exp period it runs the next chunk's
two S^T matmuls plus the previous chunk's eight PV matmuls (~0.9us).  All
eight PV accumulation groups of a slot live simultaneously in 4 PSUM banks
(two 129-column groups per 2KB bank, one shared start/stop per bank - the
start marks the whole bank pending-zero so the second group's first
accumulate overwrites cleanly).  Outputs drain PSUM -> SBUF (bf16) on the
idle Vector engine (plus Scalar for the final slot) and DMA out on the
sync queue; the host sums partials and normalizes.

The host schedules (batch, query-half, chunk-range) segments into the
slot grid with an exact-fit backtracking packer that achieves the
ceil(total/8) per-core chunk bound when possible.
"""

import math

import ml_dtypes
import numpy as np

import concourse.bacc as bacc
import concourse.mybir as mybir
import concourse.tile as tile
from concourse.bass_utils import run_bass_kernel_spmd

N_CORES = 8
B, L, D = 8, 2048, 128
CH = 128          # keys per chunk
WQ = 1024         # queries per slot
DV = D + 1        # V columns + ones column
SCALE = 1.0 / math.sqrt(D)
MASK_BIAS = -100.0

BF16 = ml_dtypes.bfloat16
FP8 = ml_dtypes.float8_e4m3


# ---------------------------------------------------------------- scheduling

def _partitions(n, max_parts):
    """Partitions of n into <= max_parts descending parts."""
    def rec(n, maxval, parts):
        if n == 0:
            yield list(parts)
            return
        if len(parts) == max_parts:
            return
        for v in range(min(n, maxval), 0, -1):
            parts.append(v)
            yield from rec(n - v, v, parts)
            parts.pop()
    yield from rec(n, n, [])


def _exact_pack(sizes, parts, budget=200000):
    """Pack groups (gid, n) into bins: 8 bins per class, class c has capacity
    parts[c].  Each bin holds at most one contiguous segment of one group.
    Returns per-class segment lists [(gid, chunk_start, seg_len), ...] or
    None.  DFS: per group choose full bins per class plus at most one
    partial bin per class."""
    counts = [8] * len(parts)
    caps = list(parts)
    nodes = [0]
    out = []  # choices per group: list of (class, seg_len) bins

    def fit(gi):
        nodes[0] += 1
        if nodes[0] > budget:
            return False
        if gi == len(sizes):
            return True
        _, n = sizes[gi]
        rem_total = sum(sz for _, sz in sizes[gi:])
        if sum(counts[c] * caps[c] for c in range(len(caps))) < rem_total:
            return False
        choice = []

        def per_class(ci, left):
            nodes[0] += 1
            if nodes[0] > budget:
                return False
            if left == 0:
                out.append(list(choice))
                if fit(gi + 1):
                    return True
                out.pop()
                return False
            if ci == len(caps):
                return False
            maxfull = min(counts[ci], left // caps[ci])
            for f in range(maxfull, -1, -1):
                rest = left - f * caps[ci]
                use_partial_opts = [0]
                if 0 < rest < caps[ci] and counts[ci] - f >= 1:
                    use_partial_opts = [rest, 0]
                for p in use_partial_opts:
                    used = f + (1 if p else 0)
                    counts[ci] -= used
                    for _ in range(f):
                        choice.append((ci, caps[ci]))
                    if p:
                        choice.append((ci, p))
                    if per_class(ci + 1, rest - p):
                        return True
                    counts[ci] += used
                    del choice[len(choice) - used:]
            return False

        return per_class(0, n)

    if not fit(0):
        return None
    per_class_segs = [[] for _ in caps]
    for (gid, _), bins in zip(sizes, out):
        done = 0
        for ci, seg in bins:
            per_class_segs[ci].append((gid, done, seg))
            done += seg
    return per_class_segs


def _greedy_pack(sizes, parts):
    """Baseline-style greedy fallback; returns per-class segs or None."""
    bins = []
    for ci, c in enumerate(parts):
        for _ in range(N_CORES):
            bins.append([c, ci])
    segs = [[] for _ in parts]
    for gid, total in sorted(sizes, key=lambda x: -x[1]):
        done = 0
        while done < total:
            rem = total - done
            if not bins:
                return None
            bins.sort(key=lambda b: b[0])
            if rem >= bins[-1][0]:
                cap, ci = bins.pop()
            else:
                i = next((i for i, b in enumerate(bins) if b[0] >= rem), None)
                if i is None:
                    return None
                cap, ci = bins.pop(i)
            take = min(cap, rem)
            segs[ci].append((gid, done, take))
            done += take
    return segs


def _schedule(valid_lens):
    """Choose a slot structure [C_1..C_S] (identical on every core) and an
    assignment of (batch, query-half) chunk segments to (core, slot)."""
    nk = [max(1, -(-int(v) // CH)) for v in valid_lens]
    groups = []  # gid -> (b, qh, nchunks)
    for b in range(B):
        for qh in range(L // WQ):
            groups.append((b, qh, nk[b]))
    sizes = sorted([(gid, g[2]) for gid, g in enumerate(groups)],
                   key=lambda x: -x[1])
    t_all = sum(s for _, s in sizes)
    tpc0 = max(1, -(-t_all // N_CORES))

    best = None  # (structure, per_class_segs)
    for tpc in range(tpc0, tpc0 + max(nk) + 2):
        for parts in _partitions(tpc, 4):
            segs = _exact_pack(sizes, parts)
            if segs is not None:
                best = (parts, segs)
                break
        if best is not None:
            break
    if best is None:
        for tpc in range(tpc0, tpc0 + 2 * max(nk) + 2):
            for parts in _partitions(tpc, 4):
                segs = _greedy_pack(sizes, parts)
                if segs is not None:
                    best = (parts, segs)
                    break
            if best is not None:
                break
    assert best is not None
    structure, per_class_segs = best

    # Distribute each class's segments to cores, balancing total load.
    S = len(structure)
    assign = [[None] * S for _ in range(N_CORES)]
    load = [0] * N_CORES
    order = sorted(range(S), key=lambda s: -max(
        [sl for _, _, sl in per_class_segs[s]] or [0]))
    for s in order:
        segs = sorted(per_class_segs[s], key=lambda x: -x[2])
        cores = sorted(range(N_CORES), key=lambda c: load[c])
        for (gid, start, seglen), core in zip(segs, cores):
            b, qh, _ = groups[gid]
            assign[core][s] = (b, qh, start, seglen)
            load[core] += seglen
    return list(structure), assign


# ------------------------------------------------------------- device program

def _slot_w(C):
    return WQ + C * (CH + DV)


def _slot_layout(structure):
    offsets = []
    base = 0
    for C in structure:
        offsets.append(base)
        base += _slot_w(C)
    return offsets, base


def _build_program(structure):
    S = len(structure)
    T = sum(structure)
    offsets, totw = _slot_layout(structure)
    slot_g0 = np.cumsum([0] + structure[:-1]).tolist()
    OUTW = 8 * DV  # 1032 output columns per slot (8 qtiles x [128 vals | den])

    nc = bacc.Bacc("TRN2", target_bir_lowering=False, debug=False)
    data_d = nc.dram_tensor("data", [128, totw], mybir.dt.bfloat16,
                            kind="ExternalInput").ap()
    bias_d = nc.dram_tensor("bias", [128, T], mybir.dt.float32,
                            kind="ExternalInput").ap()
    out_d = nc.dram_tensor("out", [128, S * OUTW], mybir.dt.bfloat16,
                           kind="ExternalOutput").ap()

    with tile.TileContext(nc) as tc:
        with tc.tile_pool(name="sb", bufs=1) as sb, \
             tc.tile_pool(name="stp", bufs=2, space="PSUM") as st_pool, \
             tc.tile_pool(name="pvp", bufs=1, space="PSUM") as pv_pool:

            # Zero the warmup operand on the (idle) Vector engine so PE
            # warmup can start right as the engines come up - GPSIMD pays a
            # ~2us library-load penalty before its first op.
            warm_sb = sb.tile([128, 512], mybir.dt.bfloat16, tag="warm")
            nc.vector.memset(warm_sb[:], 0.0)
            # Tiny dummy exp as ScalarE's first instruction: the implicit
            # 1283ns Exp-table load lands here (~6.5us, hidden behind the
            # input DMA wait) instead of right before the first real exp.
            dummy_sb = sb.tile([128, 1], mybir.dt.bfloat16, tag="dummy")
            nc.scalar.activation(dummy_sb[:], warm_sb[:, 0:1],
                                 mybir.ActivationFunctionType.Exp,
                                 bias=0.0, scale=1.0)

            data_tiles = []
            for s, C in enumerate(structure):
                w = _slot_w(C)
                d_sb = sb.tile([128, w], mybir.dt.bfloat16, tag=f"data{s}",
                               name=f"data{s}")
                data_tiles.append(d_sb)

            # Input pieces on the sync queue in consumption order: slot-0's
            # head (kt0 + qt + vx0) unblocks the first whole chunk, bias is
            # tiny and needed by the first exp, then slot-0's tail in
            # two-chunk pieces so each chunk's kt/vx lands well ahead of its
            # ~1.1us consumption slot, then the later slots whole.
            base0 = offsets[0]
            cut1 = CH + 512
            cut2 = CH + WQ + DV
            nc.sync.dma_start(data_tiles[0][:, 0:cut1],
                              data_d[:, base0:base0 + cut1])
            nc.sync.dma_start(data_tiles[0][:, cut1:cut2],
                              data_d[:, base0 + cut1:base0 + cut2])
            bias_sb = sb.tile([128, T], mybir.dt.float32, tag="bias")
            nc.sync.dma_start(bias_sb[:], bias_d[:])
            w0 = _slot_w(structure[0])
            lo = cut2
            first_tail = True
            while lo < w0:
                step = (CH + DV) if first_tail else 2 * (CH + DV)
                first_tail = False
                hi = min(w0, lo + step)
                nc.sync.dma_start(data_tiles[0][:, lo:hi],
                                  data_d[:, base0 + lo:base0 + hi])
                lo = hi
            for s in range(1, S):
                w = _slot_w(structure[s])
                base = offsets[s]
                nc.sync.dma_start(data_tiles[s][:, 0:w],
                                  data_d[:, base:base + w])

            # PE warmup: back-to-back dummy matmuls during the initial DMA
            # wait flip the HAM clock gate toward 2.4 GHz; the short trailing
            # ones keep PE busy with fine granularity so the first real
            # matmul starts within ~100ns of its data landing.
            warm_ps = st_pool.tile([128, 512], mybir.dt.float32, tag="st")
            for _ in range(6):
                nc.tensor.matmul(warm_ps[:], warm_sb[:, 0:128], warm_sb[:])
            for _ in range(8):
                nc.tensor.matmul(warm_ps[:, 0:128], warm_sb[:, 0:128],
                                 warm_sb[:, 0:128])

            def qt_ap(s):
                return data_tiles[s][:, CH:CH + WQ]

            def kt_ap(s, c):
                if c == 0:
                    return data_tiles[s][:, 0:CH]
                off = CH + WQ + DV + (c - 1) * (CH + DV)
                return data_tiles[s][:, off:off + CH]

            def vx_ap(s, c):
                if c == 0:
                    return data_tiles[s][:, CH + WQ:CH + WQ + DV]
                off = CH + WQ + DV + (c - 1) * (CH + DV) + CH
                return data_tiles[s][:, off:off + DV]

            pt_tiles = [None] * S
            pv_tiles = [None] * S

            def pt_slice(s, c):
                # pt is split into even/odd-chunk tiles so the Scalar
                # engine's exp output stream and the PE's stationary loads
                # of the previous chunk hit different SBUF regions.
                t = pt_tiles[s][c % 2]
                return t[:, (c // 2) * WQ:(c // 2 + 1) * WQ]

            def emit_pv(s, c):
                """Eight PV matmuls for chunk c of slot s.  Two accumulation
                groups share each PSUM bank; the bank's single start=True
                marks the whole 2KB zero-region pending, so the second
                group's first matmul (start=False) overwrites cleanly."""
                C = structure[s]
                if pv_tiles[s] is None:
                    pv_tiles[s] = [
                        pv_pool.tile([128, 2 * DV], mybir.dt.float32,
                                     tag=f"pv{t}", name=f"pv{t}")
                        for t in range(4)
                    ]
                pt_sb = pt_slice(s, c)
                vx = vx_ap(s, c)
                for j in range(8):
                    t, u = j // 2, j % 2
                    nc.tensor.matmul(
                        pv_tiles[s][t][:, u * DV:(u + 1) * DV],
                        pt_sb[:, j * CH:(j + 1) * CH],
                        vx,
                        start=(c == 0 and u == 0),
                        stop=(c == C - 1 and u == 1),
                    )

            def emit_outs(s, is_last=False):
                """Drain the four PV banks to an SBUF stage (bf16) and DMA
                the slot's output block out.  Steady-state drains ride the
                otherwise-idle Vector engine; the final slot also uses the
                freed Scalar engine and splits the out-DMA across two queues
                to shorten the exposed tail."""
                stage = sb.tile([128, OUTW], mybir.dt.bfloat16, tag="stage",
                                bufs=2, name="stage")
                for t in range(4):
                    dst = stage[:, t * 2 * DV:(t + 1) * 2 * DV]
                    if is_last and t % 2 == 0:
                        nc.scalar.copy(dst, pv_tiles[s][t][:])
                    else:
                        nc.vector.tensor_copy(dst, pv_tiles[s][t][:])
                half = 2 * 2 * DV
                if is_last:
                    nc.sync.dma_start(out_d[:, s * OUTW:s * OUTW + half],
                                      stage[:, 0:half])
                    nc.sync.dma_start(out_d[:, s * OUTW + half:
                                            (s + 1) * OUTW],
                                      stage[:, half:OUTW])
                else:
                    nc.sync.dma_start(out_d[:, s * OUTW:(s + 1) * OUTW],
                                      stage[:])

            prev = None
            for s, C in enumerate(structure):
                pt_tiles[s] = (
                    sb.tile([128, ((C + 1) // 2) * WQ], mybir.dt.bfloat16,
                            tag=f"pt{s}e", name=f"pt{s}e"),
                    sb.tile([128, max(1, C // 2) * WQ], mybir.dt.bfloat16,
                            tag=f"pt{s}o", name=f"pt{s}o"),
                )
                g0 = slot_g0[s]
                for c in range(C):
                    st = st_pool.tile([128, WQ], mybir.dt.float32, tag="st",
                                      name="st")
                    lhs_k = kt_ap(s, c)
                    qt = qt_ap(s)
                    nc.tensor.matmul(st[:, 0:512], lhs_k, qt[:, 0:512])
                    nc.tensor.matmul(st[:, 512:WQ], lhs_k, qt[:, 512:WQ])
                    nc.scalar.activation(pt_slice(s, c),
                                         st[:],
                                         mybir.ActivationFunctionType.Exp,
                                         bias=bias_sb[:, g0 + c:g0 + c + 1],
                                         scale=SCALE)
                    if c >= 1:
                        emit_pv(s, c - 1)
                    elif prev is not None:
                        emit_pv(prev, structure[prev] - 1)
                        emit_outs(prev)
                prev = s
            emit_pv(prev, structure[prev] - 1)
            emit_outs(prev, is_last=True)
    nc.compile()
    return nc


# ------------------------------------------------------------------- kernel

def _prep_inputs(queries, keys, values, valid_lens, structure, assign):
    T = sum(structure)
    offsets, totw = _slot_layout(structure)
    slot_g0 = np.cumsum([0] + structure[:-1]).tolist()
    karange = np.arange(CH)
    in_maps = []
    for core in range(N_CORES):
        data = np.zeros((128, totw), dtype=BF16)
        bias = np.full((128, T), MASK_BIAS, dtype=np.float32)
        for s, C in enumerate(structure):
            seg = assign[core][s]
            if seg is None:
                continue
            b, qh, cstart, ncr = seg
            base = offsets[s]
            data[:, base + CH:base + CH + WQ] = \
                queries[b, qh * WQ:(qh + 1) * WQ, :].T
            g = slot_g0[s]
            for ci in range(ncr):
                k0 = (cstart + ci) * CH
                if ci == 0:
                    kt0, vx0 = base, base + CH + WQ
                else:
                    kt0 = base + CH + WQ + DV + (ci - 1) * (CH + DV)
                    vx0 = kt0 + CH
                data[:, kt0:kt0 + CH] = keys[b, k0:k0 + CH, :].T
                data[:, vx0:vx0 + D] = values[b, k0:k0 + CH, :]
                valid = (k0 + karange) < int(valid_lens[b])
                data[:, vx0 + D] = valid
                bias[:, g + ci] = np.where(valid, 0.0, MASK_BIAS)
        in_maps.append({"data": data, "bias": bias})
    return in_maps


def _gather(results, structure, assign):
    S = len(structure)
    num = np.zeros((B, L, D), dtype=np.float64)
    den = np.zeros((B, L), dtype=np.float64)
    for core in range(N_CORES):
        out = np.asarray(results[core]["out"], dtype=np.float64)
        # [128p, S, 4t, 2u, DV] -> per-slot [q = (t*2+u)*128 + p, DV]
        out = out.reshape(128, S, 4, 2, DV)
        for s in range(S):
            seg = assign[core][s]
            if seg is None:
                continue
            b, qh, _, _ = seg
            blk = out[:, s].transpose(1, 2, 0, 3).reshape(WQ, DV)
            rows = slice(qh * WQ, (qh + 1) * WQ)
            num[b, rows, :] += blk[:, :D]
            den[b, rows] += blk[:, D]
    return (num / den[:, :, None]).astype(np.float32)


def kernel(queries, keys, values, valid_lens):
    queries = np.asarray(queries, dtype=np.float32)
    keys = np.asarray(keys, dtype=np.float32)
    values = np.asarray(values, dtype=np.float32)
    valid_lens = np.asarray(valid_lens, dtype=np.int32)

    structure, assign = _schedule(valid_lens)
    nc = _build_program(structure)
    in_maps = _prep_inputs(queries, keys, values, valid_lens, structure, assign)
    res = run_bass_kernel_spmd(nc, in_maps, core_ids=list(range(N_CORES)))
    return _gather(res.results, structure, assign)
